# revision 41
# baseline (speedup 1.0000x reference)
"""CausalFFTConv on 8 Trainium2 NeuronCores.

y[b,t,d] = sum_{s<=t} x[b,s,d] * k[t-s,d],  k[t,d] = exp(-|decay_d|*t)*cos(freq_d*t)

Equals the real part of a single complex-mode recurrence per channel:
    h[t] = z_d h[t-1] + x[t],  z_d = exp(-|a_d| + i f_d),  y = Re[h]

With chunk-local half-offset phases A(tau) = f*(tau + 1/2) and
c(tau)=cos(A), s(tau)=sin(A):
    y[t] = c(tau_t)*C[t] + s(tau_t)*S[t]
    C[t] = e^{-a} C[t-1] + c(tau_t) x[t]   (S likewise with s)
The post-multiplied quantities W_C = c*C, W_S = s*S satisfy their own
first-order recurrences with ratio multipliers:
    W_C[t] = (e^{-a} c(tau_t)/c(tau_t-1)) W_C[t-1] + c(tau_t)^2 x[t]
    y[t]   = W_C[t] + W_S[t]
which map directly onto the DVE tensor_tensor_scan instruction
(state = data0*state + data1 along the free axis) — no post-multiply
passes. The half-offset keeps s(tau) != 0 at tau=0; fp32 carries full
relative precision through small-|c| points, so the large ratios are
benign. y = W_C + W_S runs on the otherwise-idle TensorEngine as two
identity matmuls accumulating in PSUM (float32r moving operands; its
slight mantissa rounding puts the end-to-end error at ~1.2e-4 rel,
absmax ~2.7e-2 against an output scale of ~103); the ACT engine stages
PSUM->SBUF and issues the output DMAs. cc2 is derived on device as
1 - ss2 (exact identity; ~1e-7 additive kernel noise).

Chunk carries: the complex state g = C - iS rotates by e^{+i f CH}
across chunk boundaries; combined with the scan-state conversion
W = c*C this folds into 4 per-partition fused constants.

Sharding: d_model (1024) split 8 ways -> 128 channels per core = the
128 SBUF partitions. Full T per core, batch unrolled on the free axis.
"""

import sys

sys.path.insert(0, "/opt/trn_rl_repo")

from contextlib import ExitStack

import numpy as np

import concourse.bass as bass
import concourse.mybir as mybir
from concourse.bass_utils import run_bass_kernel_spmd

B, T, D = 4, 8192, 1024

# test-harness hooks (the grading harness just calls kernel(); these stay
# at their defaults there)
_RUN_KW: dict = {}
LAST_RESULT = None

NCORES = 8
DP = D // NCORES        # 128 channels per core == SBUF partitions
CH = 2048               # max chunk length along t (table/tile extent)
PRES_D_FRAC = 4         # 1/4 of sin-branch premult columns run on DVE


def _chunk_schedule():
    """(b, t0, L, first, last) per chunk; smaller chunks at the pipeline
    head (faster fill) and tail (faster drain)."""
    head = [1024, 1024, 2048, 2048, 2048]
    mid = [2048] * 4
    tail = [2048, 2048, 2048, 1024, 1024]
    out = []
    for b, pat in enumerate((head, mid, mid, tail)):
        t0 = b * T
        for j, L in enumerate(pat):
            out.append((b, t0, L, j == 0, j == len(pat) - 1))
            t0 += L
    return out


CHUNKS = _chunk_schedule()

_F32 = mybir.dt.float32
_F32R = mybir.dt.float32r
_F16 = mybir.dt.float16
_MUL = mybir.AluOpType.mult
_ADD = mybir.AluOpType.add


def _build_nc():
    nc = bass.Bass()
    xs = nc.declare_dram_parameter("xs", [DP, B * T], _F32, isOutput=False)
    ss2 = nc.declare_dram_parameter("ss2", [DP, CH], _F32, isOutput=False)
    rhoC = nc.declare_dram_parameter("rhoC", [DP, CH], _F32, isOutput=False)
    rhoS = nc.declare_dram_parameter("rhoS", [DP, CH], _F32, isOutput=False)
    # fused carry constants: Winit_C = qcc*WcEnd + qcs*WsEnd,
    #                        Winit_S = qsc*WcEnd + qss*WsEnd
    # [DP, 2]: column 0 for chunks of length 2048, column 1 for 1024
    qcc = nc.declare_dram_parameter("qcc", [DP, 2], _F32, isOutput=False)
    qcs = nc.declare_dram_parameter("qcs", [DP, 2], _F32, isOutput=False)
    qsc = nc.declare_dram_parameter("qsc", [DP, 2], _F32, isOutput=False)
    qss = nc.declare_dram_parameter("qss", [DP, 2], _F32, isOutput=False)
    ident = nc.declare_dram_parameter("ident", [DP, DP], _F32R, isOutput=False)
    ys = nc.declare_dram_parameter("ys", [DP, B * T], _F16, isOutput=True)

    nchunks = len(CHUNKS)

    with ExitStack() as ctx:
        ent = ctx.enter_context
        cc2_sb = ent(nc.sbuf_tensor([DP, CH], _F32))
        ss2_sb = ent(nc.sbuf_tensor([DP, CH], _F32))
        rhoC_sb = ent(nc.sbuf_tensor([DP, CH], _F32))
        rhoS_sb = ent(nc.sbuf_tensor([DP, CH], _F32))
        qcc_sb = ent(nc.sbuf_tensor([DP, 2], _F32))
        qcs_sb = ent(nc.sbuf_tensor([DP, 2], _F32))
        qsc_sb = ent(nc.sbuf_tensor([DP, 2], _F32))
        qss_sb = ent(nc.sbuf_tensor([DP, 2], _F32))
        xt_sb = ent(nc.sbuf_tensor([DP, 4 * CH], _F32))  # x chunk in
        uc_sb = ent(nc.sbuf_tensor([DP, 4 * CH], _F32))  # cc2*x
        us_sb = ent(nc.sbuf_tensor([DP, 4 * CH], _F32))  # ss2*x
        id_sb = ent(nc.sbuf_tensor([DP, DP], _F32R))     # identity weights
        y_sb = ent(nc.sbuf_tensor([DP, 4 * CH], _F16))   # y staging (ACT copy,
                                                         # fp16 downcast for DMA)
        wc_sb = ent(nc.sbuf_tensor([DP, 2 * CH], _F32R))  # W_C scan out
        ws_sb = ent(nc.sbuf_tensor([DP, 2 * CH], _F32R))  # W_S scan out
        ic_sb = ent(nc.sbuf_tensor([DP, 2], _F32))       # W_C initials
        is_sb = ent(nc.sbuf_tensor([DP, 2], _F32))       # W_S initials
        t0_sb = ent(nc.sbuf_tensor([DP, 1], _F32))       # carry scratch
        y_ps = ent(nc.psum_tensor([DP, 2 * CH], _F32))   # y via PE accumulate
        dma_in = ent(nc.semaphore("dma_in"))
        dma_tab = ent(nc.semaphore("dma_tab"))
        acttab = ent(nc.semaphore("acttab"))
        dma_out = ent(nc.semaphore("dma_out"))
        dve_s = ent(nc.semaphore("dve_s"))       # scan S done
        dve_c = ent(nc.semaphore("dve_c"))       # scan C done
        pe_y = ent(nc.semaphore("pe_y"))         # y (PE accumulate) done
        act_y = ent(nc.semaphore("act_y"))       # PSUM->SBUF copy done
        pool_uc = ent(nc.semaphore("pool_uc"))   # cos premult done
        cc2rdy = ent(nc.semaphore("cc2rdy"))     # cc2 = 1 - ss2 derived
        pool_us = ent(nc.semaphore("pool_us"))   # sin premult (pool part)
        block = ent(nc.Block(no_gpsimd_drain=True))

        @block.sync
        def _(sync: bass.BassEngine):
            # x0 first, SP tables interleaved into the first chunks:
            # ss2 (16), rhoC (32), carry consts (96)
            _, t00, L0, _, _ = CHUNKS[0]
            sync.dma_start(
                out=xt_sb[:, 0:L0], in_=xs[:, t00:t00 + L0]
            ).then_inc(dma_in, 16)
            sync.dma_start(out=ss2_sb[:], in_=ss2[:]).then_inc(dma_tab, 16)
            _, t01, L1, _, _ = CHUNKS[1]
            sync.dma_start(
                out=xt_sb[:, CH:CH + L1], in_=xs[:, t01:t01 + L1]
            ).then_inc(dma_in, 16)
            sync.dma_start(out=rhoC_sb[:], in_=rhoC[:]).then_inc(dma_tab, 16)
            for tab, sb in (
                (qcc, qcc_sb), (qcs, qcs_sb), (qsc, qsc_sb), (qss, qss_sb),
                (ident, id_sb),
            ):
                sync.dma_start(out=sb[:], in_=tab[:]).then_inc(dma_tab, 16)
            for k in range(2, nchunks):
                i = k % 4
                _, t0, L, _, _ = CHUNKS[k]
                if k >= 4:
                    # WAR on xt_sb[i]: premults of k-4 must be done.
                    sync.wait_ge(pool_uc, k - 3)
                    sync.wait_ge(pool_us, k - 3)
                    sync.wait_ge(dve_s, k - 3)
                sync.dma_start(
                    out=xt_sb[:, i * CH:i * CH + L],
                    in_=xs[:, t0:t0 + L],
                ).then_inc(dma_in, 16)
            # retire only after the last output DMA lands
            sync.wait_ge(dma_out, nchunks * 16)

        @block.scalar
        def _(scalar: bass.BassEngine):
            # ACT table share: rhoS (acttab 16)
            scalar.dma_start(out=rhoS_sb[:], in_=rhoS[:]).then_inc(acttab, 16)
            # output DMAs ride the idle ACT queue so they never block
            # input-DMA issuance on SP
            for k in range(nchunks):
                j = k % 2
                j4 = k % 4
                _, t0, L, _, _ = CHUNKS[k]
                scalar.wait_ge(pe_y, k + 1)
                if k >= 4:
                    # WAR on y_sb[j4]: out-DMA of k-4 must have drained
                    scalar.wait_ge(dma_out, (k - 3) * 16)
                scalar.copy(
                    out=y_sb[:, j4 * CH:j4 * CH + L],
                    in_=y_ps[:, j * CH:j * CH + L],
                ).then_inc(act_y, 1)
                # dma_start is a SEQ-level trigger: without this wait it
                # races the still-executing copy on the ACT engine pipe
                scalar.wait_ge(act_y, k + 1)
                scalar.dma_start(
                    out=ys[:, t0:t0 + L],
                    in_=y_sb[:, j4 * CH:j4 * CH + L],
                ).then_inc(dma_out, 16)

        @block.tensor
        def _(tensor: bass.BassEngine):
            tensor.wait_ge(dma_tab, 112)     # identity loaded
            for k in range(nchunks):
                i2 = k % 2
                _, t0g, L, first, last = CHUNKS[k]
                tensor.wait_ge(dve_c, k + 1)   # both scans of chunk k done
                if k >= 2:
                    # WAR: ACT copy of k-2 must have drained this PSUM half
                    tensor.wait_ge(act_y, k - 1)
                nseg = L // 512
                mm = None
                for seg in range(nseg):
                    pb = i2 * CH + seg * 512
                    wb = i2 * CH + seg * 512
                    tensor.matmul(
                        y_ps[:, pb:pb + 512],
                        id_sb[:],
                        wc_sb[:, wb:wb + 512],
                        start=True, stop=False,
                    )
                    mm = tensor.matmul(
                        y_ps[:, pb:pb + 512],
                        id_sb[:],
                        ws_sb[:, wb:wb + 512],
                        start=False, stop=True,
                    )
                mm.then_inc(pe_y, 1)

        @block.vector
        def _(vector: bass.BassEngine):
            vector.wait_ge(dma_tab, 16)     # ss2
            # cc2 = 1 - ss2 (exact identity cos^2 = 1 - sin^2; the 1e-7
            # absolute rounding acts as negligible additive kernel noise)
            vector.tensor_scalar(
                out=cc2_sb[:], in0=ss2_sb[:], scalar1=-1.0, scalar2=1.0,
                op0=_MUL, op1=_ADD,
            ).then_inc(cc2rdy, 1)
            for k in range(nchunks):
                i = k % 4
                i2 = k % 2
                _, t0g, L, first, last = CHUNKS[k]
                pd = L // 2 if k < 3 else (L * 19) // 32
                pc = L // 4 if k < 3 else 0
                xt = xt_sb[:, i * CH:i * CH + L]
                uc = uc_sb[:, i * CH:i * CH + L]
                us = us_sb[:, i * CH:i * CH + L]
                wc = wc_sb[:, i2 * CH:i2 * CH + L]
                ws = ws_sb[:, i2 * CH:i2 * CH + L]

                # DVE slice of the sin premult (bigger share during fill)
                vector.wait_ge(dma_in, (k + 1) * 16)
                # WAR on us[i]: scan S of k-4 must be done reading it
                # (same engine => implicit). Nothing cross-engine here.
                vector.tensor_tensor(
                    out=us[:, :pd], in0=xt[:, :pd],
                    in1=ss2_sb[:, :pd], op=_MUL,
                )
                if pc:
                    vector.tensor_tensor(
                        out=uc[:, :pc], in0=xt[:, :pc],
                        in1=cc2_sb[:, :pc], op=_MUL,
                    )

                init_c: float | bass.AP
                init_s: float | bass.AP
                if first:
                    init_c = 0.0
                    init_s = 0.0
                else:
                    init_c = ic_sb[:, i2:i2 + 1]
                    init_s = is_sb[:, i2:i2 + 1]

                if k == 0:
                    vector.wait_ge(acttab, 16)   # rhoS
                if k >= 2:
                    # WAR on wc/ws[i2]: PE matmuls of chunk k-2 read them
                    vector.wait_ge(pe_y, k - 1)
                vector.wait_ge(pool_us, k + 1)
                vector.tensor_tensor_scan(
                    out=ws, data0=rhoS_sb[:, :L], data1=us, initial=init_s,
                    op0=_MUL, op1=_ADD,
                ).then_inc(dve_s, 1)
                if k == 0:
                    vector.wait_ge(dma_tab, 96)  # rhoC + carry consts
                vector.wait_ge(pool_uc, k + 1)
                vector.tensor_tensor_scan(
                    out=wc, data0=rhoC_sb[:, :L], data1=uc, initial=init_c,
                    op0=_MUL, op1=_ADD,
                ).then_inc(dve_c, 1)

                if not last:
                    # carries for chunk k+1 (other parity slot); constant
                    # column by this chunk's length
                    q = 0 if L == 2048 else 1
                    j = 1 - i2
                    wce = wc_sb[:, i2 * CH + L - 1:i2 * CH + L].bitcast(_F32)
                    wse = ws_sb[:, i2 * CH + L - 1:i2 * CH + L].bitcast(_F32)
                    t0 = t0_sb[:]
                    vector.tensor_scalar_mul(
                        out=t0, in0=wse, scalar1=qcs_sb[:, q:q + 1]
                    )
                    vector.scalar_tensor_tensor(
                        out=ic_sb[:, j:j + 1], in0=wce,
                        scalar=qcc_sb[:, q:q + 1],
                        in1=t0, op0=_MUL, op1=_ADD,
                    )
                    vector.tensor_scalar_mul(
                        out=t0, in0=wce, scalar1=qsc_sb[:, q:q + 1]
                    )
                    vector.scalar_tensor_tensor(
                        out=is_sb[:, j:j + 1], in0=wse,
                        scalar=qss_sb[:, q:q + 1],
                        in1=t0, op0=_MUL, op1=_ADD,
                    )

                # y = W_C + W_S now happens on the PE via identity
                # matmuls accumulating into PSUM (see tensor block).

        @block.gpsimd
        def _(gpsimd: bass.BassEngine):
            gpsimd.wait_ge(dma_tab, 16)     # ss2
            for k in range(nchunks):
                i = k % 4
                _, t0g, L, _, _ = CHUNKS[k]
                pd = L // 2 if k < 3 else (L * 19) // 32
                pc = L // 4 if k < 3 else 0
                xt = xt_sb[:, i * CH:i * CH + L]
                uc = uc_sb[:, i * CH:i * CH + L]
                us = us_sb[:, i * CH:i * CH + L]

                gpsimd.wait_ge(dma_in, (k + 1) * 16)
                # us first: it feeds scan S, the head of the DVE chain
                # WAR on us[i, pd:]: scan S of k-4 read it
                if k >= 4:
                    gpsimd.wait_ge(dve_s, k - 3)
                gpsimd.tensor_tensor(
                    out=us[:, pd:], in0=xt[:, pd:],
                    in1=ss2_sb[:, pd:L], op=_MUL,
                ).then_inc(pool_us, 1)
                if k == 0:
                    gpsimd.wait_ge(cc2rdy, 1)       # derived cc2
                # WAR on uc[i]: scan C of chunk k-4 (its reader) done
                if k >= 4:
                    gpsimd.wait_ge(dve_c, k - 3)
                gpsimd.tensor_tensor(
                    out=uc[:, pc:], in0=xt[:, pc:],
                    in1=cc2_sb[:, pc:L], op=_MUL,
                ).then_inc(pool_uc, 1)

    return nc


def _host_tables(decay: np.ndarray, freq: np.ndarray):
    """float64 table construction, cast to fp32 at the end."""
    a = np.abs(decay.astype(np.float64))
    f = freq.astype(np.float64)
    damp = np.exp(-a)

    tau = np.arange(CH, dtype=np.float64) + 0.5
    A = f[:, None] * tau[None, :]         # [D, CH]
    c = np.cos(A)
    s = np.sin(A)
    eps = 1e-30
    c = np.where(np.abs(c) < eps, np.where(c >= 0, eps, -eps), c)
    s = np.where(np.abs(s) < eps, np.where(s >= 0, eps, -eps), s)
    # weight at tau = -1/2 (the scan-initial position)
    w0c = np.cos(-0.5 * f)
    w0s = np.sin(-0.5 * f)
    w0c = np.where(np.abs(w0c) < eps, eps, w0c)
    w0s = np.where(np.abs(w0s) < eps, np.where(w0s >= 0, eps, -eps), w0s)

    rhoC = np.empty_like(c)
    rhoS = np.empty_like(s)
    rhoC[:, 0] = damp * c[:, 0] / w0c
    rhoS[:, 0] = damp * s[:, 0] / w0s
    rhoC[:, 1:] = damp[:, None] * c[:, 1:] / c[:, :-1]
    rhoS[:, 1:] = damp[:, None] * s[:, 1:] / s[:, :-1]

    # carry: g' = e^{+i f L} g with g = C - iS =>
    #   C' = cos(fL) C + sin(fL) S ;  S' = cos(fL) S - sin(fL) C
    # C_end = Wc_end / c[L-1], S_end = Ws_end / s[L-1]
    # Winit_C = w0c * C', Winit_S = w0s * S'
    # column 0: L=2048 chunks; column 1: L=1024 chunks
    qcc = np.empty((len(f), 2))
    qcs = np.empty_like(qcc)
    qsc = np.empty_like(qcc)
    qss = np.empty_like(qcc)
    for col, L in ((0, 2048), (1, 1024)):
        rc = np.cos(f * L)
        rs = np.sin(f * L)
        qcc[:, col] = w0c * rc / c[:, L - 1]
        qcs[:, col] = w0c * rs / s[:, L - 1]
        qsc[:, col] = -w0s * rs / c[:, L - 1]
        qss[:, col] = w0s * rc / s[:, L - 1]

    f32 = np.float32
    return (
        (c * c).astype(f32), (s * s).astype(f32),
        rhoC.astype(f32), rhoS.astype(f32),
        qcc.astype(f32), qcs.astype(f32), qsc.astype(f32), qss.astype(f32),
    )


def kernel(x: np.ndarray, decay: np.ndarray, freq: np.ndarray) -> np.ndarray:
    # coerce to numpy: jax arrays silently keep float32 under .astype(f64)
    x = np.asarray(x)
    decay = np.asarray(decay)
    freq = np.asarray(freq)
    assert x.shape == (B, T, D), x.shape
    cc2, ss2, rhoC, rhoS, qcc, qcs, qsc, qss = _host_tables(decay, freq)

    # [B,T,D] -> [D, B*T] contiguous, split by core
    xt = np.ascontiguousarray(x.transpose(2, 0, 1).reshape(D, B * T))

    in_maps = []
    for cidx in range(NCORES):
        lo, hi = cidx * DP, (cidx + 1) * DP
        in_maps.append(
            {
                "xs": xt[lo:hi],
                "ss2": ss2[lo:hi],
                "rhoC": rhoC[lo:hi],
                "rhoS": rhoS[lo:hi],
                "qcc": np.ascontiguousarray(qcc[lo:hi]),
                "qcs": np.ascontiguousarray(qcs[lo:hi]),
                "qsc": np.ascontiguousarray(qsc[lo:hi]),
                "qss": np.ascontiguousarray(qss[lo:hi]),
                "ident": np.eye(DP, dtype=np.float32),
            }
        )

    nc = _build_nc()
    res = run_bass_kernel_spmd(nc, in_maps, list(range(NCORES)), **_RUN_KW)

    global LAST_RESULT
    LAST_RESULT = res
    y = np.empty((D, B * T), np.float32)  # upcast fp16 shards on assign
    for cidx in range(NCORES):
        y[cidx * DP:(cidx + 1) * DP] = res.results[cidx]["ys"]
    return np.ascontiguousarray(
        y.reshape(D, B, T).transpose(1, 2, 0)
    ).astype(x.dtype)


if __name__ == "__main__":
    rng = np.random.default_rng(0)
    x = rng.standard_normal((B, T, D)).astype(np.float32)
    decay = rng.standard_normal(D).astype(np.float32)
    freq = rng.standard_normal(D).astype(np.float32)
    y = kernel(x, decay, freq)
    print(y.shape, y.dtype, np.abs(y).mean())


# revision 44
# speedup vs baseline: 1.0063x; 1.0063x over previous
"""CausalFFTConv on 8 Trainium2 NeuronCores.

y[b,t,d] = sum_{s<=t} x[b,s,d] * k[t-s,d],  k[t,d] = exp(-|decay_d|*t)*cos(freq_d*t)

Equals the real part of a single complex-mode recurrence per channel:
    h[t] = z_d h[t-1] + x[t],  z_d = exp(-|a_d| + i f_d),  y = Re[h]

With chunk-local half-offset phases A(tau) = f*(tau + 1/2) and
c(tau)=cos(A), s(tau)=sin(A):
    y[t] = c(tau_t)*C[t] + s(tau_t)*S[t]
    C[t] = e^{-a} C[t-1] + c(tau_t) x[t]   (S likewise with s)
The post-multiplied quantities W_C = c*C, W_S = s*S satisfy their own
first-order recurrences with ratio multipliers:
    W_C[t] = (e^{-a} c(tau_t)/c(tau_t-1)) W_C[t-1] + c(tau_t)^2 x[t]
    y[t]   = W_C[t] + W_S[t]
which map directly onto the DVE tensor_tensor_scan instruction
(state = data0*state + data1 along the free axis) — no post-multiply
passes. The half-offset keeps s(tau) != 0 at tau=0; fp32 carries full
relative precision through small-|c| points, so the large ratios are
benign. y = W_C + W_S runs on the otherwise-idle TensorEngine as two
identity matmuls accumulating in PSUM (float32r moving operands; its
slight mantissa rounding puts the end-to-end error at ~1.2e-4 rel,
absmax ~2.7e-2 against an output scale of ~103); the ACT engine stages
PSUM->SBUF and issues the output DMAs. cc2 is derived on device as
1 - ss2 (exact identity; ~1e-7 additive kernel noise).

Chunk carries: the complex state g = C - iS rotates by e^{+i f CH}
across chunk boundaries; combined with the scan-state conversion
W = c*C this folds into 4 per-partition fused constants.

Sharding: d_model (1024) split 8 ways -> 128 channels per core = the
128 SBUF partitions. Full T per core, batch unrolled on the free axis.
"""

import sys

sys.path.insert(0, "/opt/trn_rl_repo")

from contextlib import ExitStack

import numpy as np

import concourse.bass as bass
import concourse.mybir as mybir
from concourse.bass_utils import run_bass_kernel_spmd

B, T, D = 4, 8192, 1024

# test-harness hooks (the grading harness just calls kernel(); these stay
# at their defaults there)
_RUN_KW: dict = {}
LAST_RESULT = None

NCORES = 8
DP = D // NCORES        # 128 channels per core == SBUF partitions
CH = 2048               # max chunk length along t (table/tile extent)
PRES_D_FRAC = 4         # 1/4 of sin-branch premult columns run on DVE


def _chunk_schedule():
    """(b, t0, L, first, last) per chunk; smaller chunks at the pipeline
    head (faster fill) and tail (faster drain)."""
    head = [1024, 1024, 2048, 2048, 2048]
    mid = [2048] * 4
    tail = [2048, 2048, 2048, 1024, 1024]
    out = []
    for b, pat in enumerate((head, mid, mid, tail)):
        t0 = b * T
        for j, L in enumerate(pat):
            out.append((b, t0, L, j == 0, j == len(pat) - 1))
            t0 += L
    return out


CHUNKS = _chunk_schedule()

_F32 = mybir.dt.float32
_F32R = mybir.dt.float32r
_MUL = mybir.AluOpType.mult
_ADD = mybir.AluOpType.add


def _build_nc():
    nc = bass.Bass()
    xs = nc.declare_dram_parameter("xs", [DP, B * T], _F32, isOutput=False)
    ss2 = nc.declare_dram_parameter("ss2", [DP, CH], _F32, isOutput=False)
    rhoC = nc.declare_dram_parameter("rhoC", [DP, CH], _F32, isOutput=False)
    rhoS = nc.declare_dram_parameter("rhoS", [DP, CH], _F32, isOutput=False)
    # fused carry constants: Winit_C = qcc*WcEnd + qcs*WsEnd,
    #                        Winit_S = qsc*WcEnd + qss*WsEnd
    # [DP, 2]: column 0 for chunks of length 2048, column 1 for 1024
    qcc = nc.declare_dram_parameter("qcc", [DP, 2], _F32, isOutput=False)
    qcs = nc.declare_dram_parameter("qcs", [DP, 2], _F32, isOutput=False)
    qsc = nc.declare_dram_parameter("qsc", [DP, 2], _F32, isOutput=False)
    qss = nc.declare_dram_parameter("qss", [DP, 2], _F32, isOutput=False)
    ident = nc.declare_dram_parameter("ident", [DP, DP], _F32R, isOutput=False)
    ys = nc.declare_dram_parameter("ys", [DP, B * T], _F32, isOutput=True)

    nchunks = len(CHUNKS)

    with ExitStack() as ctx:
        ent = ctx.enter_context
        cc2_sb = ent(nc.sbuf_tensor([DP, CH], _F32))
        ss2_sb = ent(nc.sbuf_tensor([DP, CH], _F32))
        rhoC_sb = ent(nc.sbuf_tensor([DP, CH], _F32))
        rhoS_sb = ent(nc.sbuf_tensor([DP, CH], _F32))
        qcc_sb = ent(nc.sbuf_tensor([DP, 2], _F32))
        qcs_sb = ent(nc.sbuf_tensor([DP, 2], _F32))
        qsc_sb = ent(nc.sbuf_tensor([DP, 2], _F32))
        qss_sb = ent(nc.sbuf_tensor([DP, 2], _F32))
        xt_sb = ent(nc.sbuf_tensor([DP, 4 * CH], _F32))  # x chunk in
        uc_sb = ent(nc.sbuf_tensor([DP, 4 * CH], _F32))  # cc2*x
        us_sb = ent(nc.sbuf_tensor([DP, 4 * CH], _F32))  # ss2*x
        id_sb = ent(nc.sbuf_tensor([DP, DP], _F32R))     # identity weights
        y_sb = ent(nc.sbuf_tensor([DP, 4 * CH], _F32))   # y staging (ACT copy)
        wc_sb = ent(nc.sbuf_tensor([DP, 2 * CH], _F32R))  # W_C scan out
        ws_sb = ent(nc.sbuf_tensor([DP, 2 * CH], _F32R))  # W_S scan out
        ic_sb = ent(nc.sbuf_tensor([DP, 2], _F32))       # W_C initials
        is_sb = ent(nc.sbuf_tensor([DP, 2], _F32))       # W_S initials
        t0_sb = ent(nc.sbuf_tensor([DP, 1], _F32))       # carry scratch
        y_ps = ent(nc.psum_tensor([DP, 2 * CH], _F32))   # y via PE accumulate
        dma_in = ent(nc.semaphore("dma_in"))
        dma_tab = ent(nc.semaphore("dma_tab"))
        acttab = ent(nc.semaphore("acttab"))
        dma_out = ent(nc.semaphore("dma_out"))
        dve_s = ent(nc.semaphore("dve_s"))       # scan S done
        dve_c = ent(nc.semaphore("dve_c"))       # scan C done
        pe_y = ent(nc.semaphore("pe_y"))         # y (PE accumulate) done
        act_y = ent(nc.semaphore("act_y"))       # PSUM->SBUF copy done
        pool_uc = ent(nc.semaphore("pool_uc"))   # cos premult done
        cc2rdy = ent(nc.semaphore("cc2rdy"))     # cc2 = 1 - ss2 derived
        pool_us = ent(nc.semaphore("pool_us"))   # sin premult (pool part)
        block = ent(nc.Block(no_gpsimd_drain=True))

        @block.sync
        def _(sync: bass.BassEngine):
            # x0 first, SP tables interleaved into the first chunks:
            # ss2 (16), rhoC (32), carry consts (96)
            _, t00, L0, _, _ = CHUNKS[0]
            sync.dma_start(
                out=xt_sb[:, 0:L0], in_=xs[:, t00:t00 + L0]
            ).then_inc(dma_in, 16)
            sync.dma_start(out=ss2_sb[:], in_=ss2[:]).then_inc(dma_tab, 16)
            _, t01, L1, _, _ = CHUNKS[1]
            sync.dma_start(
                out=xt_sb[:, CH:CH + L1], in_=xs[:, t01:t01 + L1]
            ).then_inc(dma_in, 16)
            sync.dma_start(out=rhoC_sb[:], in_=rhoC[:]).then_inc(dma_tab, 16)
            for tab, sb in (
                (qcc, qcc_sb), (qcs, qcs_sb), (qsc, qsc_sb), (qss, qss_sb),
                (ident, id_sb),
            ):
                sync.dma_start(out=sb[:], in_=tab[:]).then_inc(dma_tab, 16)
            for k in range(2, nchunks):
                i = k % 4
                _, t0, L, _, _ = CHUNKS[k]
                if k >= 4:
                    # WAR on xt_sb[i]: premults of k-4 must be done.
                    sync.wait_ge(pool_uc, k - 3)
                    sync.wait_ge(pool_us, k - 3)
                    sync.wait_ge(dve_s, k - 3)
                sync.dma_start(
                    out=xt_sb[:, i * CH:i * CH + L],
                    in_=xs[:, t0:t0 + L],
                ).then_inc(dma_in, 16)
            # retire only after the last output DMA lands
            sync.wait_ge(dma_out, nchunks * 16)

        @block.scalar
        def _(scalar: bass.BassEngine):
            # ACT table share: rhoS (acttab 16)
            scalar.dma_start(out=rhoS_sb[:], in_=rhoS[:]).then_inc(acttab, 16)
            # output DMAs ride the idle ACT queue so they never block
            # input-DMA issuance on SP
            for k in range(nchunks):
                j = k % 2
                j4 = k % 4
                _, t0, L, _, _ = CHUNKS[k]
                scalar.wait_ge(pe_y, k + 1)
                if k >= 4:
                    # WAR on y_sb[j4]: out-DMA of k-4 must have drained
                    scalar.wait_ge(dma_out, (k - 3) * 16)
                scalar.copy(
                    out=y_sb[:, j4 * CH:j4 * CH + L],
                    in_=y_ps[:, j * CH:j * CH + L],
                ).then_inc(act_y, 1)
                # dma_start is a SEQ-level trigger: without this wait it
                # races the still-executing copy on the ACT engine pipe
                scalar.wait_ge(act_y, k + 1)
                scalar.dma_start(
                    out=ys[:, t0:t0 + L],
                    in_=y_sb[:, j4 * CH:j4 * CH + L],
                ).then_inc(dma_out, 16)

        @block.tensor
        def _(tensor: bass.BassEngine):
            tensor.wait_ge(dma_tab, 112)     # identity loaded
            for k in range(nchunks):
                i2 = k % 2
                _, t0g, L, first, last = CHUNKS[k]
                tensor.wait_ge(dve_c, k + 1)   # both scans of chunk k done
                if k >= 2:
                    # WAR: ACT copy of k-2 must have drained this PSUM half
                    tensor.wait_ge(act_y, k - 1)
                nseg = L // 512
                mm = None
                for seg in range(nseg):
                    pb = i2 * CH + seg * 512
                    wb = i2 * CH + seg * 512
                    tensor.matmul(
                        y_ps[:, pb:pb + 512],
                        id_sb[:],
                        wc_sb[:, wb:wb + 512],
                        start=True, stop=False,
                    )
                    mm = tensor.matmul(
                        y_ps[:, pb:pb + 512],
                        id_sb[:],
                        ws_sb[:, wb:wb + 512],
                        start=False, stop=True,
                    )
                mm.then_inc(pe_y, 1)

        @block.vector
        def _(vector: bass.BassEngine):
            vector.wait_ge(dma_tab, 16)     # ss2
            # cc2 = 1 - ss2 (exact identity cos^2 = 1 - sin^2; the 1e-7
            # absolute rounding acts as negligible additive kernel noise)
            vector.tensor_scalar(
                out=cc2_sb[:], in0=ss2_sb[:], scalar1=-1.0, scalar2=1.0,
                op0=_MUL, op1=_ADD,
            ).then_inc(cc2rdy, 1)
            for k in range(nchunks):
                i = k % 4
                i2 = k % 2
                _, t0g, L, first, last = CHUNKS[k]
                pd = L // 2 if k < 3 else (L * 5) // 8
                pc = L // 4 if k < 3 else 0
                xt = xt_sb[:, i * CH:i * CH + L]
                uc = uc_sb[:, i * CH:i * CH + L]
                us = us_sb[:, i * CH:i * CH + L]
                wc = wc_sb[:, i2 * CH:i2 * CH + L]
                ws = ws_sb[:, i2 * CH:i2 * CH + L]

                # DVE slice of the sin premult (bigger share during fill)
                vector.wait_ge(dma_in, (k + 1) * 16)
                # WAR on us[i]: scan S of k-4 must be done reading it
                # (same engine => implicit). Nothing cross-engine here.
                vector.tensor_tensor(
                    out=us[:, :pd], in0=xt[:, :pd],
                    in1=ss2_sb[:, :pd], op=_MUL,
                )
                if pc:
                    vector.tensor_tensor(
                        out=uc[:, :pc], in0=xt[:, :pc],
                        in1=cc2_sb[:, :pc], op=_MUL,
                    )

                init_c: float | bass.AP
                init_s: float | bass.AP
                if first:
                    init_c = 0.0
                    init_s = 0.0
                else:
                    init_c = ic_sb[:, i2:i2 + 1]
                    init_s = is_sb[:, i2:i2 + 1]

                if k == 0:
                    vector.wait_ge(acttab, 16)   # rhoS
                if k >= 2:
                    # WAR on wc/ws[i2]: PE matmuls of chunk k-2 read them
                    vector.wait_ge(pe_y, k - 1)
                vector.wait_ge(pool_us, k + 1)
                vector.tensor_tensor_scan(
                    out=ws, data0=rhoS_sb[:, :L], data1=us, initial=init_s,
                    op0=_MUL, op1=_ADD,
                ).then_inc(dve_s, 1)
                if k == 0:
                    vector.wait_ge(dma_tab, 96)  # rhoC + carry consts
                vector.wait_ge(pool_uc, k + 1)
                vector.tensor_tensor_scan(
                    out=wc, data0=rhoC_sb[:, :L], data1=uc, initial=init_c,
                    op0=_MUL, op1=_ADD,
                ).then_inc(dve_c, 1)

                if not last:
                    # carries for chunk k+1 (other parity slot); constant
                    # column by this chunk's length
                    q = 0 if L == 2048 else 1
                    j = 1 - i2
                    wce = wc_sb[:, i2 * CH + L - 1:i2 * CH + L].bitcast(_F32)
                    wse = ws_sb[:, i2 * CH + L - 1:i2 * CH + L].bitcast(_F32)
                    t0 = t0_sb[:]
                    vector.tensor_scalar_mul(
                        out=t0, in0=wse, scalar1=qcs_sb[:, q:q + 1]
                    )
                    vector.scalar_tensor_tensor(
                        out=ic_sb[:, j:j + 1], in0=wce,
                        scalar=qcc_sb[:, q:q + 1],
                        in1=t0, op0=_MUL, op1=_ADD,
                    )
                    vector.tensor_scalar_mul(
                        out=t0, in0=wce, scalar1=qsc_sb[:, q:q + 1]
                    )
                    vector.scalar_tensor_tensor(
                        out=is_sb[:, j:j + 1], in0=wse,
                        scalar=qss_sb[:, q:q + 1],
                        in1=t0, op0=_MUL, op1=_ADD,
                    )

                # y = W_C + W_S now happens on the PE via identity
                # matmuls accumulating into PSUM (see tensor block).

        @block.gpsimd
        def _(gpsimd: bass.BassEngine):
            gpsimd.wait_ge(dma_tab, 16)     # ss2
            for k in range(nchunks):
                i = k % 4
                _, t0g, L, _, _ = CHUNKS[k]
                pd = L // 2 if k < 3 else (L * 5) // 8
                pc = L // 4 if k < 3 else 0
                xt = xt_sb[:, i * CH:i * CH + L]
                uc = uc_sb[:, i * CH:i * CH + L]
                us = us_sb[:, i * CH:i * CH + L]

                gpsimd.wait_ge(dma_in, (k + 1) * 16)
                # us first: it feeds scan S, the head of the DVE chain
                # WAR on us[i, pd:]: scan S of k-4 read it
                if k >= 4:
                    gpsimd.wait_ge(dve_s, k - 3)
                gpsimd.tensor_tensor(
                    out=us[:, pd:], in0=xt[:, pd:],
                    in1=ss2_sb[:, pd:L], op=_MUL,
                ).then_inc(pool_us, 1)
                if k == 0:
                    gpsimd.wait_ge(cc2rdy, 1)       # derived cc2
                # WAR on uc[i]: scan C of chunk k-4 (its reader) done
                if k >= 4:
                    gpsimd.wait_ge(dve_c, k - 3)
                gpsimd.tensor_tensor(
                    out=uc[:, pc:], in0=xt[:, pc:],
                    in1=cc2_sb[:, pc:L], op=_MUL,
                ).then_inc(pool_uc, 1)

    return nc


def _host_tables(decay: np.ndarray, freq: np.ndarray):
    """float64 table construction, cast to fp32 at the end."""
    a = np.abs(decay.astype(np.float64))
    f = freq.astype(np.float64)
    damp = np.exp(-a)

    tau = np.arange(CH, dtype=np.float64) + 0.5
    A = f[:, None] * tau[None, :]         # [D, CH]
    c = np.cos(A)
    s = np.sin(A)
    eps = 1e-30
    c = np.where(np.abs(c) < eps, np.where(c >= 0, eps, -eps), c)
    s = np.where(np.abs(s) < eps, np.where(s >= 0, eps, -eps), s)
    # weight at tau = -1/2 (the scan-initial position)
    w0c = np.cos(-0.5 * f)
    w0s = np.sin(-0.5 * f)
    w0c = np.where(np.abs(w0c) < eps, eps, w0c)
    w0s = np.where(np.abs(w0s) < eps, np.where(w0s >= 0, eps, -eps), w0s)

    rhoC = np.empty_like(c)
    rhoS = np.empty_like(s)
    rhoC[:, 0] = damp * c[:, 0] / w0c
    rhoS[:, 0] = damp * s[:, 0] / w0s
    rhoC[:, 1:] = damp[:, None] * c[:, 1:] / c[:, :-1]
    rhoS[:, 1:] = damp[:, None] * s[:, 1:] / s[:, :-1]

    # carry: g' = e^{+i f L} g with g = C - iS =>
    #   C' = cos(fL) C + sin(fL) S ;  S' = cos(fL) S - sin(fL) C
    # C_end = Wc_end / c[L-1], S_end = Ws_end / s[L-1]
    # Winit_C = w0c * C', Winit_S = w0s * S'
    # column 0: L=2048 chunks; column 1: L=1024 chunks
    qcc = np.empty((len(f), 2))
    qcs = np.empty_like(qcc)
    qsc = np.empty_like(qcc)
    qss = np.empty_like(qcc)
    for col, L in ((0, 2048), (1, 1024)):
        rc = np.cos(f * L)
        rs = np.sin(f * L)
        qcc[:, col] = w0c * rc / c[:, L - 1]
        qcs[:, col] = w0c * rs / s[:, L - 1]
        qsc[:, col] = -w0s * rs / c[:, L - 1]
        qss[:, col] = w0s * rc / s[:, L - 1]

    f32 = np.float32
    return (
        (c * c).astype(f32), (s * s).astype(f32),
        rhoC.astype(f32), rhoS.astype(f32),
        qcc.astype(f32), qcs.astype(f32), qsc.astype(f32), qss.astype(f32),
    )


def kernel(x: np.ndarray, decay: np.ndarray, freq: np.ndarray) -> np.ndarray:
    # coerce to numpy: jax arrays silently keep float32 under .astype(f64)
    x = np.asarray(x)
    decay = np.asarray(decay)
    freq = np.asarray(freq)
    assert x.shape == (B, T, D), x.shape
    cc2, ss2, rhoC, rhoS, qcc, qcs, qsc, qss = _host_tables(decay, freq)

    # [B,T,D] -> [D, B*T] contiguous, split by core
    xt = np.ascontiguousarray(x.transpose(2, 0, 1).reshape(D, B * T))

    in_maps = []
    for cidx in range(NCORES):
        lo, hi = cidx * DP, (cidx + 1) * DP
        in_maps.append(
            {
                "xs": xt[lo:hi],
                "ss2": ss2[lo:hi],
                "rhoC": rhoC[lo:hi],
                "rhoS": rhoS[lo:hi],
                "qcc": np.ascontiguousarray(qcc[lo:hi]),
                "qcs": np.ascontiguousarray(qcs[lo:hi]),
                "qsc": np.ascontiguousarray(qsc[lo:hi]),
                "qss": np.ascontiguousarray(qss[lo:hi]),
                "ident": np.eye(DP, dtype=np.float32),
            }
        )

    nc = _build_nc()
    res = run_bass_kernel_spmd(nc, in_maps, list(range(NCORES)), **_RUN_KW)

    global LAST_RESULT
    LAST_RESULT = res
    y = np.empty((D, B * T), np.float32)
    for cidx in range(NCORES):
        y[cidx * DP:(cidx + 1) * DP] = res.results[cidx]["ys"]
    return np.ascontiguousarray(
        y.reshape(D, B, T).transpose(1, 2, 0)
    ).astype(x.dtype)


if __name__ == "__main__":
    rng = np.random.default_rng(0)
    x = rng.standard_normal((B, T, D)).astype(np.float32)
    decay = rng.standard_normal(D).astype(np.float32)
    freq = rng.standard_normal(D).astype(np.float32)
    y = kernel(x, decay, freq)
    print(y.shape, y.dtype, np.abs(y).mean())
